# revision 6
# baseline (speedup 1.0000x reference)
"""Trainium2 Bass kernel for ContrastDrivenFeatureAggregation.

Computation (per batch image, C=64 channels, H=W=128, HEADS=4, HEAD_DIM=16,
3x3 window):
    v = x_l @ W_v + b_v                               (1x1 conv)
    attn[h, t, l] = sum_i (cat(fg,bg)_l @ [W_fg;W_bg] + b)[h, i, t]
    out[c, l] = sum_t attn[h(c), t, l] * v[c, l + delta_t]   (zero-padded)

Host-side algebra: the sum over i is folded into the weights
(Wsum = W.reshape(128, 4, 9, 9).sum(axis=2)), and the 36-row attention map is
computed directly in channel-replicated form (64 rows, one per channel) by
replicating weight columns, so the device never broadcasts across partitions.

Sharding: pure data parallel over 8 cores = 4 batches x 2 image halves
(rows 0-63 / 64-127), with a 1-row x halo for the 3x3 aggregation.

Device layout: 128 SBUF partitions = 64 channels x 2 half-shards (rows 0-31 /
32-63 of the shard).  Per tap: PE computes replicated attention into PSUM
(fp32r), DVE does fused (attn + bias) * v_shifted, PE accumulates taps into a
PSUM output bank via identity matmuls.  The x-boundary wrap of the flat shift
is masked by dropping the wrapped column in the accumulation matmul's access
pattern.
"""

import sys

for _p in ("/opt/trn_rl_repo",):
    if _p not in sys.path:
        sys.path.insert(0, _p)

import numpy as np

import concourse.bass as bass
import concourse.mybir as mybir
import concourse.tile as tile
from concourse.bass_utils import run_bass_kernel_spmd

B, C, H, W = 4, 64, 128, 128
HEADS, HD = 4, 16
K, KK = 3, 9
N_CORES = 8

ROWS = 64          # image rows per shard
HALF = 32          # rows per partition-half
LH = HALF * W      # pixels per half = 4096
VROWS = HALF + 2   # v rows per half incl halo
VF = VROWS * W     # 4352
XROWS = ROWS + 2   # x rows incl halo
XF = XROWS * W     # 8448
NSC = 4            # superchunks
SCW = LH // NSC    # 1024 cols per half per superchunk

F32 = mybir.dt.float32
F32R = mybir.dt.float32r

# tap order for accumulation: a dx==0 tap first so the first matmul in the
# accumulation group initializes every output column
TAP_ORDER = [4, 1, 7, 0, 2, 3, 5, 6, 8]

_COMPILED = None


def _r(ap):
    return ap.bitcast(F32R)


def _build():
    nc = bass.Bass("TRN2", num_devices=N_CORES)

    x_d = nc.dram_tensor("x_sh", [C + 1, XF], F32, kind="ExternalInput")
    g_d = nc.dram_tensor("g_sh", [2 * C, ROWS * W], F32, kind="ExternalInput")
    wv_d = nc.dram_tensor("wv", [C + 1, C], F32, kind="ExternalInput")
    wr_d = nc.dram_tensor("wrep", [2 * C, KK * C], F32, kind="ExternalInput")
    br_d = nc.dram_tensor("brep", [2 * C, KK], F32, kind="ExternalInput")
    id_d = nc.dram_tensor("ident", [2 * C, 2 * C], F32, kind="ExternalInput")
    out_d = nc.dram_tensor("out_sh", [2 * C, LH], F32, kind="ExternalOutput")

    with tile.TileContext(nc) as tc:
        with (
            tc.tile_pool(name="cpool", bufs=1) as cpool,
            tc.tile_pool(name="prodp", bufs=3) as prodp,
            tc.tile_pool(name="outp", bufs=2) as outp,
            tc.tile_pool(name="vps", bufs=2, space="PSUM") as vps,
            tc.tile_pool(name="aps", bufs=2, space="PSUM") as aps,
            tc.tile_pool(name="ops", bufs=1, space="PSUM") as ops,
        ):
            xt = cpool.tile([C + 1, XF], F32)
            gt = cpool.tile([2 * C, ROWS * W], F32)
            wvt = cpool.tile([C + 1, C], F32)
            wrt = cpool.tile([2 * C, KK * C], F32)
            brt = cpool.tile([2 * C, KK], F32)
            idt = cpool.tile([2 * C, 2 * C], F32)
            # v tile with one spare column on each side: the corner taps of
            # boundary pixels read 1 element past the data (x-wrap garbage,
            # masked out in the accumulation matmuls)
            vt = cpool.tile([2 * C, VF + 2], F32)
            nc.vector.memset(vt[:, 0:1], 0.0)
            nc.vector.memset(vt[:, VF + 1 : VF + 2], 0.0)

            nc.sync.dma_start(xt[:], x_d[:])
            nc.sync.dma_start(gt[:], g_d[:])
            nc.sync.dma_start(wvt[:], wv_d[:])
            nc.sync.dma_start(wrt[:], wr_d[:])
            nc.sync.dma_start(brt[:], br_d[:])
            nc.sync.dma_start(idt[:], id_d[:])

            # ---- v = [x; rowmask] @ [W_v; b_v], both halves (34 rows each)
            nv = (VF + 511) // 512  # 9 chunks (last is 256)
            for k in range(nv):
                cw = min(512, VF - k * 512)
                vp = vps.tile([2 * C, 512], F32, tag="vp")
                nc.tensor.matmul(
                    vp[0:C, 0:cw],
                    _r(wvt[:]),
                    _r(xt[:, k * 512 : k * 512 + cw]),
                    start=True, stop=True,
                )
                nc.tensor.matmul(
                    vp[C : 2 * C, 0:cw],
                    _r(wvt[:]),
                    _r(xt[:, LH + k * 512 : LH + k * 512 + cw]),
                    start=True, stop=True,
                    tile_position=(0, 64),
                )
                nc.scalar.copy(vt[:, 1 + k * 512 : 1 + k * 512 + cw], vp[:, 0:cw])

            # ---- main loop: per superchunk, per tap
            for sc in range(NSC):
                op = ops.tile([2 * C, SCW], F32, tag="op")
                for jn, j in enumerate(TAP_ORDER):
                    dy, dx = j // 3 - 1, j % 3 - 1
                    at = aps.tile([2 * C, SCW], F32, tag="at")
                    lhs = wrt[:, j * C : (j + 1) * C]
                    for h2 in range(SCW // 512):
                        c0 = sc * SCW + h2 * 512
                        nc.tensor.matmul(
                            at[0:C, h2 * 512 : h2 * 512 + 512],
                            _r(lhs), _r(gt[:, c0 : c0 + 512]),
                            start=True, stop=True,
                        )
                        nc.tensor.matmul(
                            at[C : 2 * C, h2 * 512 : h2 * 512 + 512],
                            _r(lhs), _r(gt[:, LH + c0 : LH + c0 + 512]),
                            start=True, stop=True,
                            tile_position=(0, 64),
                        )
                    prod = prodp.tile([2 * C, SCW], F32, tag="prod")
                    voff = 1 + sc * SCW + (1 + dy) * W + dx
                    nc.vector.scalar_tensor_tensor(
                        prod[:],
                        at[:],
                        brt[:, j : j + 1],
                        vt[:, voff : voff + SCW],
                        op0=mybir.AluOpType.add,
                        op1=mybir.AluOpType.mult,
                    )
                    # the flat shift wraps at x image boundaries for dx != 0:
                    # zero those product columns (true contribution is 0 since
                    # the reference zero-pads v)
                    if dx != 0:
                        xo = 0 if dx == -1 else W - 1
                        pr = prod.rearrange("p (r x) -> p r x", x=W)
                        nc.vector.memset(pr[:, :, xo : xo + 1], 0.0)
                    # accumulate into output PSUM
                    first, last = jn == 0, jn == len(TAP_ORDER) - 1
                    for h2 in range(SCW // 512):
                        s0 = h2 * 512
                        nc.tensor.matmul(
                            op[:, s0 : s0 + 512], _r(idt[:]),
                            _r(prod[:, s0 : s0 + 512]),
                            start=first, stop=last,
                            skip_group_check=True,
                        )
                outt = outp.tile([2 * C, SCW], F32, tag="outt")
                nc.scalar.copy(outt[:], op[:])
                nc.sync.dma_start(out_d[:, sc * SCW : (sc + 1) * SCW], outt[:])

    return nc


def _get_compiled():
    global _COMPILED
    if _COMPILED is None:
        _COMPILED = _build()
    return _COMPILED


def _prep_shards(x, fg, bg, W_v, b_v, W_fg, b_fg, W_bg, b_bg):
    x = np.asarray(x, np.float32)
    fg = np.asarray(fg, np.float32)
    bg = np.asarray(bg, np.float32)

    Wcat = np.concatenate([np.asarray(W_fg, np.float32),
                           np.asarray(W_bg, np.float32)], axis=0)  # (128, 324)
    bcat = np.asarray(b_fg, np.float32) + np.asarray(b_bg, np.float32)
    Wsum = Wcat.reshape(2 * C, HEADS, KK, KK).sum(axis=2)  # (128, 4, 9) [q,h,t]
    bsum = bcat.reshape(HEADS, KK, KK).sum(axis=1)         # (4, 9)  [h,t]

    # wrep[q, t, c] = Wsum[q, h(c), t]
    wrep = np.repeat(Wsum[:, :, None, :], HD, axis=2)      # (128, 4, 16, 9)
    wrep = wrep.transpose(0, 3, 1, 2).reshape(2 * C, KK * C).copy()
    brep64 = np.repeat(bsum, HD, axis=0)                   # (64, 9) [c,t]
    brep = np.concatenate([brep64, brep64], axis=0)        # (128, 9)

    wv = np.concatenate([np.asarray(W_v, np.float32),
                         np.asarray(b_v, np.float32)[None, :]], axis=0)
    ident = np.eye(2 * C, dtype=np.float32)

    in_maps = []
    for core in range(N_CORES):
        b, half = core // 2, core % 2
        r0 = half * ROWS
        # x shard with 1-row halo on both sides + in-image indicator channel
        xs = np.zeros((C + 1, XROWS, W), np.float32)
        lo, hi = r0 - 1, r0 + ROWS + 1
        clo, chi = max(lo, 0), min(hi, H)
        xs[:C, clo - lo : clo - lo + (chi - clo), :] = x[b, :, clo:chi, :]
        rows = np.arange(lo, hi)
        xs[C, :, :] = ((rows >= 0) & (rows < H)).astype(np.float32)[:, None]
        gs = np.concatenate(
            [fg[b, :, r0 : r0 + ROWS, :], bg[b, :, r0 : r0 + ROWS, :]], axis=0
        )
        in_maps.append({
            "x_sh": xs.reshape(C + 1, XF).copy(),
            "g_sh": gs.reshape(2 * C, ROWS * W).copy(),
            "wv": wv.copy(),
            "wrep": wrep.copy(),
            "brep": brep.copy(),
            "ident": ident.copy(),
        })
    return in_maps


def _assemble(results):
    out = np.empty((B, C, H, W), np.float32)
    for core, res in enumerate(results):
        b, half = core // 2, core % 2
        r0 = half * ROWS
        o = res["out_sh"]  # (128, 4096)
        out[b, :, r0 : r0 + HALF, :] = o[:C].reshape(C, HALF, W)
        out[b, :, r0 + HALF : r0 + ROWS, :] = o[C:].reshape(C, HALF, W)
    return out


def kernel(**inputs) -> np.ndarray:
    nc = _get_compiled()
    in_maps = _prep_shards(**inputs)
    res = run_bass_kernel_spmd(nc, in_maps, core_ids=list(range(N_CORES)))
    return _assemble(res.results)


if __name__ == "__main__":
    # smoke test against a local reference
    sys.path.insert(0, "/root/problem")
    import reference

    inputs = {k: np.asarray(v) for k, v in reference.setup_inputs().items()}
    expected = np.asarray(reference.reference(**inputs))
    actual = kernel(**inputs)
    err = np.abs(actual - expected)
    denom = np.abs(expected).max()
    print("max abs err:", err.max(), "rel:", err.max() / denom)
